# revision 6
# baseline (speedup 1.0000x reference)
"""Masked FFN kernel for trn2 (8 NeuronCores, SPMD data-parallel over rows).

Math: out = (gelu(x @ W1 + b1) @ W2 + b2) * mask  with masked-out rows exactly 0.

Strategy:
  - Host compacts the (B*T) rows down to the ~50% active ones (mask != 0),
    shards them evenly across 8 cores, pads per-core row count R to a
    block-friendly capacity.
  - Device computes the FFN on compacted rows only, in transposed layout:
      mm1: H^T[f, r] = sum_d W1[d, f] * X^T[d, r]   (W1 tile stationary)
      gelu+b1 fused on ScalarE (PSUM -> SBUF)
      mm2: Y^T[o, r] = sum_f W2[f, o] * H^T[f, r]   (W2 tile stationary)
      +b2 fused on ScalarE (PSUM -> SBUF), DMA out.
    Both matmuls run as float32r (full PE rate, fp32 storage).
  - Host scatters Y rows back into a zero output.
"""

import numpy as np

import concourse.bass as bass
import concourse.tile as tile
from concourse import bacc, mybir
from concourse import bass_utils

N_CORES = 8
D = 1024      # model dim
F = 4096      # ffn dim
DT = D // 128  # 8 d-tiles
FT = F // 128  # 32 f-tiles
OT = D // 128  # 8 output tiles

F32 = mybir.dt.float32
F32R = mybir.dt.float32r

_CACHE: dict = {}
LAST_RESULTS = None  # BassKernelResults of the most recent device run (for test harness)


def _blocks(rc: int):
    """Split rc into blocks of size in [256, 512] (except rc < 256 -> [rc])."""
    if rc <= 512:
        return [(0, rc)]
    out = []
    pos, rem = 0, rc
    while rem > 512:
        b = min(512, rem - 256)
        out.append((pos, b))
        pos += b
        rem -= b
    out.append((pos, rem))
    return out


def _build(rc: int, nch: int):
    key = (rc, nch)
    if key in _CACHE:
        return _CACHE[key]

    nc = bacc.Bacc("TRN2", target_bir_lowering=False, debug=False, num_devices=N_CORES,
                   dynamic_dma_scratch_size=8192)
    xt_d = nc.dram_tensor("xt", [nch, DT, 128, rc], F32, kind="ExternalInput").ap()
    w1_d = nc.dram_tensor("w1t", [FT, DT, 128, 128], F32, kind="ExternalInput").ap()
    b1_d = nc.dram_tensor("b1m", [128, FT], F32, kind="ExternalInput").ap()
    w2_d = nc.dram_tensor("w2t", [OT, FT, 128, 128], F32, kind="ExternalInput").ap()
    b2_d = nc.dram_tensor("b2m", [128, OT], F32, kind="ExternalInput").ap()
    yt_d = nc.dram_tensor("yt", [nch, OT, 128, rc], F32, kind="ExternalOutput").ap()

    blocks = _blocks(rc)
    gelu = mybir.ActivationFunctionType.Gelu_apprx_tanh
    ident = mybir.ActivationFunctionType.Identity

    with tile.TileContext(nc) as tc:
        with (
            tc.tile_pool(name="consts", bufs=1) as consts,
            tc.tile_pool(name="xpool", bufs=DT) as xpool,
            tc.tile_pool(name="hpool", bufs=1) as hpool,
            tc.tile_pool(name="w1pool", bufs=3) as w1pool,
            tc.tile_pool(name="w2pool", bufs=6) as w2pool,
            tc.tile_pool(name="ypool", bufs=3) as ypool,
            tc.tile_pool(name="pspool", bufs=6, space="PSUM") as pspool,
        ):
            b1_sb = consts.tile([128, FT], F32)
            nc.sync.dma_start(out=b1_sb, in_=b1_d)
            b2_sb = consts.tile([128, OT], F32)
            nc.sync.dma_start(out=b2_sb, in_=b2_d)

            for ch in range(nch):
                xt_sb = []
                for d_t in range(DT):
                    xd = xpool.tile([128, rc], F32, tag="xt")
                    nc.sync.dma_start(
                        out=xd[:].bitcast(F32R),
                        in_=xt_d[ch, d_t].bitcast(F32R),
                    )
                    xt_sb.append(xd)
                ht_sb = hpool.tile([128, FT, rc], F32, tag="ht")

                # ---- phase A: H^T = gelu(W1^T-tiles @ X^T + b1) ----
                for f_t in range(FT):
                    w1sb = w1pool.tile([128, DT, 128], F32, tag="w1")
                    nc.scalar.dma_start(
                        out=w1sb[:].bitcast(F32R),
                        in_=w1_d[f_t].rearrange("a p f -> p a f").bitcast(F32R),
                    )
                    for b0, blk in blocks:
                        ps = pspool.tile([128, 512], F32, tag="ps")
                        for d_t in range(DT):
                            nc.tensor.matmul(
                                ps[:, :blk],
                                lhsT=w1sb[:, d_t, :].bitcast(F32R),
                                rhs=xt_sb[d_t][:, b0 : b0 + blk].bitcast(F32R),
                                start=(d_t == 0),
                                stop=(d_t == DT - 1),
                            )
                        nc.scalar.activation(
                            out=ht_sb[:, f_t, b0 : b0 + blk].bitcast(F32R),
                            in_=ps[:, :blk],
                            func=gelu,
                            bias=b1_sb[:, f_t : f_t + 1],
                            scale=1.0,
                        )

                # ---- phase B: Y^T = W2^T-tiles @ H^T + b2 ----
                for o_t in range(OT):
                    w2h = []
                    for h in range(4):
                        w = w2pool.tile([128, FT // 4, 128], F32, tag="w2")
                        nc.scalar.dma_start(
                            out=w[:].bitcast(F32R),
                            in_=w2_d[o_t, h * (FT // 4) : (h + 1) * (FT // 4)].rearrange(
                                "a p f -> p a f"
                            ).bitcast(F32R),
                        )
                        w2h.append(w)
                    for b0, blk in blocks:
                        ps2 = pspool.tile([128, 512], F32, tag="ps")
                        for f_t in range(FT):
                            nc.tensor.matmul(
                                ps2[:, :blk],
                                lhsT=w2h[f_t // (FT // 4)][:, f_t % (FT // 4), :].bitcast(F32R),
                                rhs=ht_sb[:, f_t, b0 : b0 + blk].bitcast(F32R),
                                start=(f_t == 0),
                                stop=(f_t == FT - 1),
                            )
                        yt_t = ypool.tile([128, 512], F32, tag="yt")
                        nc.scalar.activation(
                            out=yt_t[:, :blk],
                            in_=ps2[:, :blk],
                            func=ident,
                            bias=b2_sb[:, o_t : o_t + 1],
                            scale=1.0,
                        )
                        nc.sync.dma_start(
                            out=yt_d[ch, o_t, :, b0 : b0 + blk], in_=yt_t[:, :blk]
                        )

    nc.compile()
    _CACHE[key] = nc
    return nc


def _pick_shape(r_need: int):
    """Choose (rc, nch) given required per-core rows."""
    rc_max = 1088
    nch = 1
    while True:
        rc = -(-r_need // nch)          # ceil
        rc = max(256, -(-rc // 64) * 64)  # round up to 64, floor 256
        if rc <= rc_max:
            return rc, nch
        nch += 1


def kernel(inputs: np.ndarray, mask: np.ndarray, W1: np.ndarray, b1: np.ndarray,
           W2: np.ndarray, b2: np.ndarray) -> np.ndarray:
    global LAST_RESULTS
    B, T, Dm = inputs.shape
    assert Dm == D and W1.shape == (D, F) and W2.shape == (F, D)
    N = B * T

    x_flat = np.ascontiguousarray(np.asarray(inputs, dtype=np.float32).reshape(N, D))
    m_flat = np.asarray(mask).reshape(N).astype(bool)
    idx = np.flatnonzero(m_flat)
    na = idx.size
    out = np.zeros((N, D), dtype=np.float32)
    if na == 0:
        return out.reshape(B, T, D)

    r_need = -(-na // N_CORES)
    rc, nch = _pick_shape(r_need)
    cap = rc * nch

    nc = None
    while nc is None:
        try:
            nc = _build(rc, nch)
        except AssertionError:
            # SBUF overflow at this rc -> split into more chunks
            nch += 1
            rc = max(256, -(-(-(-r_need // nch)) // 64) * 64)
            cap = rc * nch

    idx_pad = np.zeros(N_CORES * cap, dtype=np.int64)
    idx_pad[:na] = idx
    xg = x_flat[idx_pad]  # [N_CORES*cap, D]

    # weight/bias tilings (shared by all cores)
    w1t = np.ascontiguousarray(
        np.asarray(W1, np.float32).reshape(DT, 128, FT, 128).transpose(2, 0, 1, 3)
    )
    w2t = np.ascontiguousarray(
        np.asarray(W2, np.float32).reshape(FT, 128, OT, 128).transpose(2, 0, 1, 3)
    )
    b1m = np.ascontiguousarray(np.asarray(b1, np.float32).reshape(FT, 128).T)
    b2m = np.ascontiguousarray(np.asarray(b2, np.float32).reshape(OT, 128).T)

    in_maps = []
    for c in range(N_CORES):
        xc = xg[c * cap : (c + 1) * cap]  # [cap, D]
        xt = np.empty((nch, DT, 128, rc), dtype=np.float32)
        for ch in range(nch):
            xt[ch] = xc[ch * rc : (ch + 1) * rc].T.reshape(DT, 128, rc)
        in_maps.append({"xt": xt, "w1t": w1t, "b1m": b1m, "w2t": w2t, "b2m": b2m})

    res = bass_utils.run_bass_kernel_spmd(nc, in_maps, core_ids=list(range(N_CORES)))
    LAST_RESULTS = res

    y_parts = []
    for c in range(N_CORES):
        yt = res.results[c]["yt"]  # [nch, OT, 128, rc]
        for ch in range(nch):
            y_parts.append(yt[ch].reshape(D, rc).T)  # [rc, D]
    ycat = np.concatenate(y_parts, axis=0)  # [N_CORES*cap, D]
    out[idx] = ycat[:na]
    return out.reshape(B, T, D)


# revision 8
# speedup vs baseline: 1.1940x; 1.1940x over previous
"""Masked FFN kernel for trn2 (8 NeuronCores, SPMD data-parallel over rows).

Math: out = (gelu(x @ W1 + b1) @ W2 + b2) * mask  with masked-out rows exactly 0.

Strategy:
  - Host compacts the (B*T) rows down to the ~50% active ones (mask != 0),
    shards them evenly across 8 cores, pads per-core row count R to a
    block-friendly capacity.
  - Device computes the FFN on compacted rows only, in transposed layout:
      mm1: H^T[f, r] = sum_d W1[d, f] * X^T[d, r]   (W1 tile stationary)
      gelu+b1 fused on ScalarE (PSUM -> SBUF)
      mm2: Y^T[o, r] = sum_f W2[f, o] * H^T[f, r]   (W2 tile stationary)
      +b2 fused on ScalarE (PSUM -> SBUF), DMA out.
    Both matmuls run as float32r (full PE rate, fp32 storage).
  - Host scatters Y rows back into a zero output.
"""

import numpy as np

import concourse.bass as bass
import concourse.tile as tile
from concourse import bacc, mybir
from concourse import bass_utils

N_CORES = 8
D = 1024      # model dim
F = 4096      # ffn dim
DT = D // 128  # 8 d-tiles
FT = F // 128  # 32 f-tiles
OT = D // 128  # 8 output tiles

F32 = mybir.dt.float32
F32R = mybir.dt.float32r

_CACHE: dict = {}
LAST_RESULTS = None  # BassKernelResults of the most recent device run (for test harness)


def _blocks(rc: int):
    """Split rc into blocks of size in [256, 512] (except rc < 256 -> [rc])."""
    if rc <= 512:
        return [(0, rc)]
    out = []
    pos, rem = 0, rc
    while rem > 512:
        b = min(512, rem - 256)
        out.append((pos, b))
        pos += b
        rem -= b
    out.append((pos, rem))
    return out


def _build(rc: int, nch: int):
    key = (rc, nch)
    if key in _CACHE:
        return _CACHE[key]

    nc = bacc.Bacc("TRN2", target_bir_lowering=False, debug=False, num_devices=N_CORES,
                   dynamic_dma_scratch_size=8192)
    xt_d = nc.dram_tensor("xt", [nch, DT, 128, rc], F32, kind="ExternalInput").ap()
    w1_d = nc.dram_tensor("w1t", [FT, DT, 128, 128], F32, kind="ExternalInput").ap()
    b1_d = nc.dram_tensor("b1m", [128, FT], F32, kind="ExternalInput").ap()
    w2_d = nc.dram_tensor("w2t", [OT, FT, 128, 128], F32, kind="ExternalInput").ap()
    b2_d = nc.dram_tensor("b2m", [128, OT], F32, kind="ExternalInput").ap()
    yt_d = nc.dram_tensor("yt", [nch, OT, 128, rc], F32, kind="ExternalOutput").ap()

    blocks = _blocks(rc)
    gelu = mybir.ActivationFunctionType.Gelu_apprx_tanh
    ident = mybir.ActivationFunctionType.Identity

    with tile.TileContext(nc) as tc:
        with (
            tc.tile_pool(name="consts", bufs=1) as consts,
            tc.tile_pool(name="xpool", bufs=DT) as xpool,
            tc.tile_pool(name="hpool", bufs=1) as hpool,
            tc.tile_pool(name="w1pool", bufs=3) as w1pool,
            tc.tile_pool(name="w2pool", bufs=6) as w2pool,
            tc.tile_pool(name="ypool", bufs=3) as ypool,
            tc.tile_pool(name="pspool", bufs=6, space="PSUM") as pspool,
        ):
            b1_sb = consts.tile([128, FT], F32)
            nc.sync.dma_start(out=b1_sb, in_=b1_d)
            b2_sb = consts.tile([128, OT], F32)
            nc.sync.dma_start(out=b2_sb, in_=b2_d)

            for ch in range(nch):
                # first f-tile's W1 slab goes first in the DMA queue so the
                # PE can start as soon as the first x block lands
                w1_first = w1pool.tile([128, DT, 128], F32, tag="w1")
                nc.sync.dma_start(
                    out=w1_first[:].bitcast(F32R),
                    in_=w1_d[0].rearrange("a p f -> p a f").bitcast(F32R),
                )
                xt_sb = {}
                for bi, (b0, blk) in enumerate(blocks):
                    for d_t in range(DT):
                        xd = xpool.tile([128, blk], F32, tag=f"xt{bi}")
                        nc.sync.dma_start(
                            out=xd[:].bitcast(F32R),
                            in_=xt_d[ch, d_t, :, b0 : b0 + blk].bitcast(F32R),
                        )
                        xt_sb[(d_t, bi)] = xd
                ht_sb = hpool.tile([128, FT, rc], F32, tag="ht")

                # ---- phase A: H^T = gelu(W1^T-tiles @ X^T + b1) ----
                for f_t in range(FT):
                    if f_t == 0:
                        w1sb = w1_first
                    else:
                        w1sb = w1pool.tile([128, DT, 128], F32, tag="w1")
                        nc.sync.dma_start(
                            out=w1sb[:].bitcast(F32R),
                            in_=w1_d[f_t].rearrange("a p f -> p a f").bitcast(F32R),
                        )
                    for bi, (b0, blk) in enumerate(blocks):
                        ps = pspool.tile([128, 512], F32, tag="ps")
                        for d_t in range(DT):
                            nc.tensor.matmul(
                                ps[:, :blk],
                                lhsT=w1sb[:, d_t, :].bitcast(F32R),
                                rhs=xt_sb[(d_t, bi)][:].bitcast(F32R),
                                start=(d_t == 0),
                                stop=(d_t == DT - 1),
                            )
                        nc.scalar.activation(
                            out=ht_sb[:, f_t, b0 : b0 + blk].bitcast(F32R),
                            in_=ps[:, :blk],
                            func=gelu,
                            bias=b1_sb[:, f_t : f_t + 1],
                            scale=1.0,
                        )

                # ---- phase B: Y^T = W2^T-tiles @ H^T + b2 ----
                for o_t in range(OT):
                    w2h = []
                    for h in range(4):
                        w = w2pool.tile([128, FT // 4, 128], F32, tag="w2")
                        nc.sync.dma_start(
                            out=w[:].bitcast(F32R),
                            in_=w2_d[o_t, h * (FT // 4) : (h + 1) * (FT // 4)].rearrange(
                                "a p f -> p a f"
                            ).bitcast(F32R),
                        )
                        w2h.append(w)
                    for b0, blk in blocks:
                        ps2 = pspool.tile([128, 512], F32, tag="ps")
                        for f_t in range(FT):
                            nc.tensor.matmul(
                                ps2[:, :blk],
                                lhsT=w2h[f_t // (FT // 4)][:, f_t % (FT // 4), :].bitcast(F32R),
                                rhs=ht_sb[:, f_t, b0 : b0 + blk].bitcast(F32R),
                                start=(f_t == 0),
                                stop=(f_t == FT - 1),
                            )
                        yt_t = ypool.tile([128, 512], F32, tag="yt")
                        nc.scalar.activation(
                            out=yt_t[:, :blk],
                            in_=ps2[:, :blk],
                            func=ident,
                            bias=b2_sb[:, o_t : o_t + 1],
                            scale=1.0,
                        )
                        nc.sync.dma_start(
                            out=yt_d[ch, o_t, :, b0 : b0 + blk], in_=yt_t[:, :blk]
                        )

    nc.compile()
    _CACHE[key] = nc
    return nc


def _pick_shape(r_need: int):
    """Choose (rc, nch) given required per-core rows."""
    rc_max = 1088
    nch = 1
    while True:
        rc = -(-r_need // nch)          # ceil
        rc = max(256, -(-rc // 64) * 64)  # round up to 64, floor 256
        if rc <= rc_max:
            return rc, nch
        nch += 1


def kernel(inputs: np.ndarray, mask: np.ndarray, W1: np.ndarray, b1: np.ndarray,
           W2: np.ndarray, b2: np.ndarray) -> np.ndarray:
    global LAST_RESULTS
    B, T, Dm = inputs.shape
    assert Dm == D and W1.shape == (D, F) and W2.shape == (F, D)
    N = B * T

    x_flat = np.ascontiguousarray(np.asarray(inputs, dtype=np.float32).reshape(N, D))
    m_flat = np.asarray(mask).reshape(N).astype(bool)
    idx = np.flatnonzero(m_flat)
    na = idx.size
    out = np.zeros((N, D), dtype=np.float32)
    if na == 0:
        return out.reshape(B, T, D)

    r_need = -(-na // N_CORES)
    rc, nch = _pick_shape(r_need)
    cap = rc * nch

    nc = None
    while nc is None:
        try:
            nc = _build(rc, nch)
        except AssertionError:
            # SBUF overflow at this rc -> split into more chunks
            nch += 1
            rc = max(256, -(-(-(-r_need // nch)) // 64) * 64)
            cap = rc * nch

    idx_pad = np.zeros(N_CORES * cap, dtype=np.int64)
    idx_pad[:na] = idx
    xg = x_flat[idx_pad]  # [N_CORES*cap, D]

    # weight/bias tilings (shared by all cores)
    w1t = np.ascontiguousarray(
        np.asarray(W1, np.float32).reshape(DT, 128, FT, 128).transpose(2, 0, 1, 3)
    )
    w2t = np.ascontiguousarray(
        np.asarray(W2, np.float32).reshape(FT, 128, OT, 128).transpose(2, 0, 1, 3)
    )
    b1m = np.ascontiguousarray(np.asarray(b1, np.float32).reshape(FT, 128).T)
    b2m = np.ascontiguousarray(np.asarray(b2, np.float32).reshape(OT, 128).T)

    in_maps = []
    for c in range(N_CORES):
        xc = xg[c * cap : (c + 1) * cap]  # [cap, D]
        xt = np.empty((nch, DT, 128, rc), dtype=np.float32)
        for ch in range(nch):
            xt[ch] = xc[ch * rc : (ch + 1) * rc].T.reshape(DT, 128, rc)
        in_maps.append({"xt": xt, "w1t": w1t, "b1m": b1m, "w2t": w2t, "b2m": b2m})

    res = bass_utils.run_bass_kernel_spmd(nc, in_maps, core_ids=list(range(N_CORES)))
    LAST_RESULTS = res

    y_parts = []
    for c in range(N_CORES):
        yt = res.results[c]["yt"]  # [nch, OT, 128, rc]
        for ch in range(nch):
            y_parts.append(yt[ch].reshape(D, rc).T)  # [rc, D]
    ycat = np.concatenate(y_parts, axis=0)  # [N_CORES*cap, D]
    out[idx] = ycat[:na]
    return out.reshape(B, T, D)
